# revision 28
# baseline (speedup 1.0000x reference)
"""Trainium2 Bass kernel for nn_AffinityImageEvent.

Math: the reference L2-normalizes image/event over C, then for each of the
9 offsets (i,j) of a 3x3 window computes sum_c img_shift*evt_shift -- both
tensors shifted by the SAME offset.  That means every output channel k is
just a shifted copy of the zero-padded per-pixel cosine map

    D[b,h,w] = (img . evt) / (||img|| ||evt||)        (over C=128)
    out[b, k=(i,j), h, w] = relu(Dpad[b, h+i, w+j])

So the kernel computes three C-reductions per pixel (img.evt, img^2, evt^2),
a tiny pointwise epilogue, and 9 shifted DMA stores.

Sharding: B(4) x H-halves(2) -> 8 cores, each core gets [C=128, 98, 256]
(96 rows + 1 halo row each side, zero-padded at image boundaries).

Per-core pipeline (DMA-bound target ~72us):
  - stream input in 14-row chunks [128, 3584] f32 via HWDGE
  - elementwise products split across three engines to stay under the DMA
    shadow: prod=img*evt (GpSimd/DVE), img^2 and evt^2 (DVE/ACT) written
    interleaved per row into one [128, 2*3584] fp32r tile
  - PE: per row r, sliding one-hot ones-column lhsT (A[:, 98-r:196-r], col
    r hot) reduces [128C, 256W] into PSUM partition r.  s1 uses an exact
    fp32 matmul (cancellation-sensitive); s2|s3 share one full-rate fp32r
    matmul [128, 512] -> one PSUM bank.
  - epilogue: D = relu(s1) * rsqrt(s2*s3 + eps)  (sqrt on ACT, reciprocal
    on DVE, one Newton step), written into a zero-framed [98, 258] tile
  - 9 shifted DMAs [96, 256] -> out[k]
"""

import sys

sys.path.insert(0, "/opt/trn_rl_repo")

import numpy as np

try:
    import jax

    jax.config.update("jax_compilation_cache_dir", "/tmp/affinity_jaxcache")
    jax.config.update("jax_persistent_cache_min_compile_time_secs", 1.0)
    jax.config.update("jax_persistent_cache_min_entry_size_bytes", 0)
except Exception:
    pass

import concourse.bass as bass
import concourse.bacc as bacc
import concourse.tile as tile
from concourse import mybir
from concourse.bass_utils import run_bass_kernel_spmd

B, C, H, W = 4, 128, 192, 256
KWIN = 9
N_CORES = 8
HALF = H // 2              # 96 output rows per core
ROWS = HALF + 2            # 98 D rows incl. halo
# chunk row-count schedule: small first chunk gets PE started early;
# tapered last chunks drain the product/matmul pipeline quickly after the
# final DMA so the epilogue starts sooner.
CHUNK_SCHED = [2] + [7] * 12 + [5, 4, 2, 1]
assert sum(CHUNK_SCHED) == ROWS
MAXCW = max(CHUNK_SCHED) * W
IN_BUFS = 3
PROD_BUFS = 3

F32 = mybir.dt.float32
F32R = mybir.dt.float32r
AF = mybir.ActivationFunctionType

# knobs
S1_DTYPE = F32             # img.evt reduction (cancellation-prone -> exact)
NEWTON = False             # rsqrt via ACT Sqrt + DVE reciprocal is already
                           # fp32-accurate on HW (measured); Newton step off
EPS = 1e-30                # keeps zero halo rows finite (0 * big = 0)


def build_program(repeat: int = 1) -> bass.Bass:
    nc = bacc.Bacc("TRN2", target_bir_lowering=False, debug=False)
    img_d = nc.dram_tensor("image", [C, ROWS * W], F32, kind="ExternalInput").ap()
    evt_d = nc.dram_tensor("event", [C, ROWS * W], F32, kind="ExternalInput").ap()
    out_d = nc.dram_tensor("out", [KWIN, HALF, W], F32, kind="ExternalOutput").ap()

    # sliding one-hot: A[:, 98-r : 196-r] has its ones-column at position r
    # -> matmul writes row-r sums to PSUM partition r.  Raw sbuf tensors,
    # initialized inside the TileContext (deps tracked; Bacc legalizes any
    # multi-wait instructions into event-semaphore NOPs).
    A32 = nc.alloc_sbuf_tensor("onehot_f32", [C, 2 * ROWS], F32).ap()
    A32r = nc.alloc_sbuf_tensor("onehot", [C, 2 * ROWS], F32R).ap()

    with tile.TileContext(nc) as tc:
        with (
            tc.tile_pool(name="inp", bufs=IN_BUFS) as ipool,
            tc.tile_pool(name="prod", bufs=PROD_BUFS) as ppool,
            tc.tile_pool(name="acc", bufs=2, space="PSUM") as psum,
            tc.tile_pool(name="epi", bufs=2) as epool,
        ):
            nc.gpsimd.memset(A32[:, 0:ROWS], 0.0)
            nc.gpsimd.memset(A32[:, ROWS : ROWS + 1], 1.0)
            nc.gpsimd.memset(A32[:, ROWS + 1 : 2 * ROWS], 0.0)
            nc.vector.tensor_copy(A32r, A32)
            eps_t = nc.alloc_sbuf_tensor("eps", [C, 1], F32).ap()
            nc.gpsimd.memset(eps_t, EPS)
            # dummy Sqrt up front nudges the act-table pass to load a
            # sqrt-capable set (sqrt_and_* also contain Square/Relu/Copy),
            # avoiding a mid-epilogue table switch
            warm_t = nc.alloc_sbuf_tensor("actwarm", [C, 1], F32).ap()
            nc.scalar.activation(warm_t, eps_t, AF.Sqrt)

            for _ in range(repeat):
                s1 = psum.tile([C, W], F32, tag="s1")
                s23 = psum.tile([C, 2 * W], F32, tag="s23")

                r0 = 0
                for k, crows in enumerate(CHUNK_SCHED):
                    cw = crows * W
                    cs0 = r0 * W
                    img_t = ipool.tile([C, MAXCW], F32, tag="img")
                    nc.sync.dma_start(
                        out=img_t[:, 0:cw], in_=img_d[:, cs0 : cs0 + cw]
                    )
                    evt_t = ipool.tile([C, MAXCW], F32, tag="evt")
                    nc.sync.dma_start(
                        out=evt_t[:, 0:cw], in_=evt_d[:, cs0 : cs0 + cw]
                    )

                    # prod rows contiguous (f32 for exact s1 matmul)
                    prod = ppool.tile([C, MAXCW], S1_DTYPE, tag="prod")
                    nc.vector.tensor_mul(
                        prod[:, 0:cw], img_t[:, 0:cw], evt_t[:, 0:cw]
                    )

                    # squares interleaved per row: [sqi_row | sqe_row] so one
                    # fp32r matmul covers both norms
                    sq = ppool.tile([C, 2 * MAXCW], F32R, tag="sq")
                    sqv = sq[:, 0 : 2 * cw].rearrange(
                        "c (q x) -> c q x", x=2 * W
                    )
                    img3 = img_t[:, 0:cw].rearrange("c (q w) -> c q w", w=W)
                    evt3 = evt_t[:, 0:cw].rearrange("c (q w) -> c q w", w=W)
                    nc.scalar.activation(sqv[:, :, 0:W], img3, AF.Square)
                    nc.scalar.activation(sqv[:, :, W : 2 * W], evt3, AF.Square)

                    for q in range(crows):
                        r = r0 + q
                        st = r == 0
                        sp = r == ROWS - 1
                        nc.tensor.matmul(
                            s1[0:ROWS, :],
                            A32[:, ROWS - r : 2 * ROWS - r],
                            prod[:, q * W : (q + 1) * W],
                            start=st,
                            stop=sp,
                        )
                        nc.tensor.matmul(
                            s23[0:ROWS, :],
                            A32r[:, ROWS - r : 2 * ROWS - r],
                            sq[:, q * 2 * W : (q + 1) * 2 * W],
                            start=st,
                            stop=sp,
                        )
                    r0 += crows

                # epilogue: D = relu(s1) * rsqrt(s2*s3 + eps)
                rp = slice(0, ROWS)
                s2sb = epool.tile([C, W], F32, tag="s2sb")
                nc.scalar.activation(s2sb[rp, :], s23[rp, 0:W], AF.Copy)
                t23 = epool.tile([C, W], F32, tag="t23")
                nc.vector.tensor_mul(t23[rp, :], s2sb[rp, :], s23[rp, W : 2 * W])
                sqr = epool.tile([C, W], F32, tag="sqr")
                nc.scalar.activation(
                    sqr[rp, :], t23[rp, :], AF.Sqrt, bias=eps_t[rp]
                )
                y = epool.tile([C, W], F32, tag="y")
                nc.vector.reciprocal(y[rp, :], sqr[rp, :])
                if NEWTON:
                    # y' = y * (1.5 - 0.5 * t * y^2); t23 without eps is fine:
                    # zero rows give t23=0 -> w=1.5, y stays finite.
                    y2 = epool.tile([C, W], F32, tag="y2")
                    nc.vector.tensor_mul(y2[rp, :], y[rp, :], y[rp, :])
                    h = epool.tile([C, W], F32, tag="h")
                    nc.vector.tensor_mul(h[rp, :], y2[rp, :], t23[rp, :])
                    w_ = epool.tile([C, W], F32, tag="w_")
                    nc.vector.tensor_scalar(
                        w_[rp, :], h[rp, :], -0.5, 1.5,
                        mybir.AluOpType.mult, mybir.AluOpType.add,
                    )
                    yn = epool.tile([C, W], F32, tag="yn")
                    nc.vector.tensor_mul(yn[rp, :], y[rp, :], w_[rp, :])
                    y = yn
                s1r = epool.tile([C, W], F32, tag="s1r")
                nc.scalar.activation(s1r[rp, :], s1[rp, :], AF.Relu)

                dpad = epool.tile([C, W + 2], F32, tag="dpad")
                nc.vector.memset(dpad[rp, 0:1], 0.0)
                nc.vector.memset(dpad[rp, W + 1 : W + 2], 0.0)
                nc.vector.tensor_mul(dpad[rp, 1 : W + 1], s1r[rp, :], y[rp, :])

                # one DMA per window-row i covers the 3 j-shifts: SBUF side
                # reads overlapping [96, 3, 256] windows; DRAM side reorders
                # to (h, k, w) via transpose.
                out4 = out_d.rearrange("(i j) h w -> i j h w", i=3)
                for i in range(3):
                    src = dpad[i : i + HALF, 0:W]
                    sap = src.ap
                    src3 = bass.AP(
                        src.tensor,
                        src.offset,
                        [list(sap[0]), [1, 3], list(sap[1])],
                    )
                    dst3 = out4[i].transpose([1, 0, 2])
                    eng = nc.sync if i != 1 else nc.scalar
                    eng.dma_start(out=dst3, in_=src3)
    nc.finalize()
    return nc


def _make_shards(image: np.ndarray, event: np.ndarray):
    in_maps = []
    for c in range(N_CORES):
        b, half = divmod(c, 2)
        h0 = half * HALF
        m = {}
        for name, src in (("image", image), ("event", event)):
            shard = np.zeros((C, ROWS, W), dtype=np.float32)
            r0 = max(h0 - 1, 0)
            r1 = min(h0 + HALF + 1, H)
            d0 = r0 - (h0 - 1)
            shard[:, d0 : d0 + (r1 - r0), :] = src[b, :, r0:r1, :]
            m[name] = shard.reshape(C, ROWS * W)
        in_maps.append(m)
    return in_maps


_PROGRAM = None


def _get_program():
    global _PROGRAM
    if _PROGRAM is None:
        _PROGRAM = build_program()
    return _PROGRAM


def run(image: np.ndarray, event: np.ndarray, trace: bool = False):
    """Run on 8 cores; returns (full_output, BassKernelResults)."""
    image = np.ascontiguousarray(np.asarray(image), dtype=np.float32)
    event = np.ascontiguousarray(np.asarray(event), dtype=np.float32)
    assert image.shape == (B, C, H, W) and event.shape == (B, C, H, W)
    nc = _get_program()
    in_maps = _make_shards(image, event)
    res = run_bass_kernel_spmd(nc, in_maps, list(range(N_CORES)), trace=trace)
    full = np.empty((B, KWIN, H, W), dtype=np.float32)
    for c in range(N_CORES):
        b, half = divmod(c, 2)
        h0 = half * HALF
        full[b, :, h0 : h0 + HALF, :] = res.results[c]["out"]
    return full, res


def kernel(image: np.ndarray, event: np.ndarray) -> np.ndarray:
    out, _ = run(image, event, trace=False)
    return out


# revision 30
# speedup vs baseline: 1.5187x; 1.5187x over previous
"""Trainium2 Bass kernel for nn_AffinityImageEvent.

Math: the reference L2-normalizes image/event over C, then for each of the
9 offsets (i,j) of a 3x3 window computes sum_c img_shift*evt_shift -- both
tensors shifted by the SAME offset.  That means every output channel k is
just a shifted copy of the zero-padded per-pixel cosine map

    D[b,h,w] = (img . evt) / (||img|| ||evt||)        (over C=128)
    out[b, k=(i,j), h, w] = relu(Dpad[b, h+i, w+j])

So the kernel computes three C-reductions per pixel (img.evt, img^2, evt^2),
a tiny pointwise epilogue, and 9 shifted DMA stores.

Sharding: B(4) x H-halves(2) -> 8 cores, each core gets [C=128, 98, 256]
(96 rows + 1 halo row each side, zero-padded at image boundaries).

Per-core pipeline (DMA-bound target ~72us):
  - stream input in 14-row chunks [128, 3584] f32 via HWDGE
  - elementwise products split across three engines to stay under the DMA
    shadow: prod=img*evt (GpSimd/DVE), img^2 and evt^2 (DVE/ACT) written
    interleaved per row into one [128, 2*3584] fp32r tile
  - PE: per row r, sliding one-hot ones-column lhsT (A[:, 98-r:196-r], col
    r hot) reduces [128C, 256W] into PSUM partition r.  s1 uses an exact
    fp32 matmul (cancellation-sensitive); s2|s3 share one full-rate fp32r
    matmul [128, 512] -> one PSUM bank.
  - epilogue: D = relu(s1) * rsqrt(s2*s3 + eps)  (sqrt on ACT, reciprocal
    on DVE, one Newton step), written into a zero-framed [98, 258] tile
  - 9 shifted DMAs [96, 256] -> out[k]
"""

import sys

sys.path.insert(0, "/opt/trn_rl_repo")

import numpy as np

try:
    import jax

    jax.config.update("jax_compilation_cache_dir", "/tmp/affinity_jaxcache")
    jax.config.update("jax_persistent_cache_min_compile_time_secs", 1.0)
    jax.config.update("jax_persistent_cache_min_entry_size_bytes", 0)
except Exception:
    pass

import concourse.bass as bass
import concourse.bacc as bacc
import concourse.tile as tile
from concourse import mybir
from concourse.bass_utils import run_bass_kernel_spmd

B, C, H, W = 4, 128, 192, 256
KWIN = 9
N_CORES = 8
HALF = H // 2              # 96 output rows per core
ROWS = HALF + 2            # 98 D rows incl. halo
# chunk row-count schedule: small first chunk gets PE started early; a
# gradual taper at the end collapses the products->matmul pipeline phase
# lag (~ one chunk of work) so the epilogue starts right after the last DMA.
CHUNK_SCHED = [2] + [7] * 10 + [6, 5, 4, 3, 2, 2, 2, 1, 1]
assert sum(CHUNK_SCHED) == ROWS
MAXCW = max(CHUNK_SCHED) * W
IN_BUFS = 3
PROD_BUFS = 3

F32 = mybir.dt.float32
F32R = mybir.dt.float32r
AF = mybir.ActivationFunctionType

# knobs
S1_DTYPE = F32             # img.evt reduction (cancellation-prone -> exact)
NEWTON = False             # rsqrt via ACT Sqrt + DVE reciprocal is already
                           # fp32-accurate on HW (measured); Newton step off
EPS = 1e-30                # keeps zero halo rows finite (0 * big = 0)


def build_program(repeat: int = 1) -> bass.Bass:
    nc = bacc.Bacc("TRN2", target_bir_lowering=False, debug=False)
    img_d = nc.dram_tensor("image", [C, ROWS * W], F32, kind="ExternalInput").ap()
    evt_d = nc.dram_tensor("event", [C, ROWS * W], F32, kind="ExternalInput").ap()
    out_d = nc.dram_tensor("out", [KWIN, HALF, W], F32, kind="ExternalOutput").ap()

    # sliding one-hot: A[:, 98-r : 196-r] has its ones-column at position r
    # -> matmul writes row-r sums to PSUM partition r.  Raw sbuf tensors,
    # initialized inside the TileContext (deps tracked; Bacc legalizes any
    # multi-wait instructions into event-semaphore NOPs).
    A32 = nc.alloc_sbuf_tensor("onehot_f32", [C, 2 * ROWS], F32).ap()
    A32r = nc.alloc_sbuf_tensor("onehot", [C, 2 * ROWS], F32R).ap()

    with tile.TileContext(nc) as tc:
        with (
            tc.tile_pool(name="inp", bufs=IN_BUFS) as ipool,
            tc.tile_pool(name="prod", bufs=PROD_BUFS) as ppool,
            tc.tile_pool(name="acc", bufs=2, space="PSUM") as psum,
            tc.tile_pool(name="epi", bufs=2) as epool,
        ):
            nc.gpsimd.memset(A32[:, 0:ROWS], 0.0)
            nc.gpsimd.memset(A32[:, ROWS : ROWS + 1], 1.0)
            nc.gpsimd.memset(A32[:, ROWS + 1 : 2 * ROWS], 0.0)
            nc.vector.tensor_copy(A32r, A32)
            eps_t = nc.alloc_sbuf_tensor("eps", [C, 1], F32).ap()
            nc.gpsimd.memset(eps_t, EPS)
            # dummy Sqrt up front nudges the act-table pass to load a
            # sqrt-capable set (sqrt_and_* also contain Square/Relu/Copy),
            # avoiding a mid-epilogue table switch
            warm_t = nc.alloc_sbuf_tensor("actwarm", [C, 1], F32).ap()
            nc.scalar.activation(warm_t, eps_t, AF.Sqrt)

            for _ in range(repeat):
                s1 = psum.tile([C, W], F32, tag="s1")
                s23 = psum.tile([C, 2 * W], F32, tag="s23")

                r0 = 0
                for k, crows in enumerate(CHUNK_SCHED):
                    cw = crows * W
                    cs0 = r0 * W
                    img_t = ipool.tile([C, MAXCW], F32, tag="img")
                    nc.sync.dma_start(
                        out=img_t[:, 0:cw], in_=img_d[:, cs0 : cs0 + cw]
                    )
                    evt_t = ipool.tile([C, MAXCW], F32, tag="evt")
                    nc.sync.dma_start(
                        out=evt_t[:, 0:cw], in_=evt_d[:, cs0 : cs0 + cw]
                    )

                    # prod rows contiguous (f32 for exact s1 matmul)
                    prod = ppool.tile([C, MAXCW], S1_DTYPE, tag="prod")
                    nc.vector.tensor_mul(
                        prod[:, 0:cw], img_t[:, 0:cw], evt_t[:, 0:cw]
                    )

                    # squares interleaved per row: [sqi_row | sqe_row] so one
                    # fp32r matmul covers both norms
                    sq = ppool.tile([C, 2 * MAXCW], F32R, tag="sq")
                    sqv = sq[:, 0 : 2 * cw].rearrange(
                        "c (q x) -> c q x", x=2 * W
                    )
                    img3 = img_t[:, 0:cw].rearrange("c (q w) -> c q w", w=W)
                    evt3 = evt_t[:, 0:cw].rearrange("c (q w) -> c q w", w=W)
                    if crows < 7:
                        # taper region: split squares DVE || ACT to halve
                        # the pipeline-drain latency
                        nc.vector.tensor_mul(sqv[:, :, 0:W], img3, img3)
                    else:
                        nc.scalar.activation(sqv[:, :, 0:W], img3, AF.Square)
                    nc.scalar.activation(sqv[:, :, W : 2 * W], evt3, AF.Square)

                    for q in range(crows):
                        r = r0 + q
                        st = r == 0
                        sp = r == ROWS - 1
                        nc.tensor.matmul(
                            s1[0:ROWS, :],
                            A32[:, ROWS - r : 2 * ROWS - r],
                            prod[:, q * W : (q + 1) * W],
                            start=st,
                            stop=sp,
                        )
                        nc.tensor.matmul(
                            s23[0:ROWS, :],
                            A32r[:, ROWS - r : 2 * ROWS - r],
                            sq[:, q * 2 * W : (q + 1) * 2 * W],
                            start=st,
                            stop=sp,
                        )
                    r0 += crows

                # epilogue: D = relu(s1) * rsqrt(s2*s3 + eps)
                rp = slice(0, ROWS)
                s2sb = epool.tile([C, W], F32, tag="s2sb")
                nc.scalar.activation(s2sb[rp, :], s23[rp, 0:W], AF.Copy)
                t23 = epool.tile([C, W], F32, tag="t23")
                nc.vector.tensor_mul(t23[rp, :], s2sb[rp, :], s23[rp, W : 2 * W])
                sqr = epool.tile([C, W], F32, tag="sqr")
                nc.scalar.activation(
                    sqr[rp, :], t23[rp, :], AF.Sqrt, bias=eps_t[rp]
                )
                y = epool.tile([C, W], F32, tag="y")
                nc.vector.reciprocal(y[rp, :], sqr[rp, :])
                if NEWTON:
                    # y' = y * (1.5 - 0.5 * t * y^2); t23 without eps is fine:
                    # zero rows give t23=0 -> w=1.5, y stays finite.
                    y2 = epool.tile([C, W], F32, tag="y2")
                    nc.vector.tensor_mul(y2[rp, :], y[rp, :], y[rp, :])
                    h = epool.tile([C, W], F32, tag="h")
                    nc.vector.tensor_mul(h[rp, :], y2[rp, :], t23[rp, :])
                    w_ = epool.tile([C, W], F32, tag="w_")
                    nc.vector.tensor_scalar(
                        w_[rp, :], h[rp, :], -0.5, 1.5,
                        mybir.AluOpType.mult, mybir.AluOpType.add,
                    )
                    yn = epool.tile([C, W], F32, tag="yn")
                    nc.vector.tensor_mul(yn[rp, :], y[rp, :], w_[rp, :])
                    y = yn
                s1r = epool.tile([C, W], F32, tag="s1r")
                nc.scalar.activation(s1r[rp, :], s1[rp, :], AF.Relu)

                dpad = epool.tile([C, W + 2], F32, tag="dpad")
                nc.vector.memset(dpad[rp, 0:1], 0.0)
                nc.vector.memset(dpad[rp, W + 1 : W + 2], 0.0)
                nc.vector.tensor_mul(dpad[rp, 1 : W + 1], s1r[rp, :], y[rp, :])

                # one DMA per window-row i covers the 3 j-shifts: SBUF side
                # reads overlapping [96, 3, 256] windows; DRAM side reorders
                # to (h, k, w) via transpose.
                out4 = out_d.rearrange("(i j) h w -> i j h w", i=3)
                for i in range(3):
                    src = dpad[i : i + HALF, 0:W]
                    sap = src.ap
                    src3 = bass.AP(
                        src.tensor,
                        src.offset,
                        [list(sap[0]), [1, 3], list(sap[1])],
                    )
                    dst3 = out4[i].transpose([1, 0, 2])
                    eng = nc.sync if i != 1 else nc.scalar
                    eng.dma_start(out=dst3, in_=src3)
    nc.finalize()
    return nc


def _make_shards(image: np.ndarray, event: np.ndarray):
    in_maps = []
    for c in range(N_CORES):
        b, half = divmod(c, 2)
        h0 = half * HALF
        m = {}
        for name, src in (("image", image), ("event", event)):
            shard = np.zeros((C, ROWS, W), dtype=np.float32)
            r0 = max(h0 - 1, 0)
            r1 = min(h0 + HALF + 1, H)
            d0 = r0 - (h0 - 1)
            shard[:, d0 : d0 + (r1 - r0), :] = src[b, :, r0:r1, :]
            m[name] = shard.reshape(C, ROWS * W)
        in_maps.append(m)
    return in_maps


_PROGRAM = None


def _get_program():
    global _PROGRAM
    if _PROGRAM is None:
        _PROGRAM = build_program()
    return _PROGRAM


def run(image: np.ndarray, event: np.ndarray, trace: bool = False):
    """Run on 8 cores; returns (full_output, BassKernelResults)."""
    image = np.ascontiguousarray(np.asarray(image), dtype=np.float32)
    event = np.ascontiguousarray(np.asarray(event), dtype=np.float32)
    assert image.shape == (B, C, H, W) and event.shape == (B, C, H, W)
    nc = _get_program()
    in_maps = _make_shards(image, event)
    res = run_bass_kernel_spmd(nc, in_maps, list(range(N_CORES)), trace=trace)
    full = np.empty((B, KWIN, H, W), dtype=np.float32)
    for c in range(N_CORES):
        b, half = divmod(c, 2)
        h0 = half * HALF
        full[b, :, h0 : h0 + HALF, :] = res.results[c]["out"]
    return full, res


def kernel(image: np.ndarray, event: np.ndarray) -> np.ndarray:
    out, _ = run(image, event, trace=False)
    return out
